# revision 4
# baseline (speedup 1.0000x reference)
"""Causal self-attention (B=4, T=2048, C=1024, H=16) on 8 TRN2 NeuronCores.

Sharding: core c -> (batch b = c//2, head-group g = c%2 of 8 heads).
Each core computes its batch's QKV projection for its 8 heads, causal
attention, and a partial output projection (row-parallel W_proj slice).
Host sums the two partial projections per batch (unshard of the
row-parallel linear).

All matmuls run in fp32r (fp32 rounded to 11 mantissa bits, 4x the fp32
matmul rate on the PE). Intermediates consumed by matmuls are written as
float32r so the walrus verifier sees rounded producers.

Device-side layout avoids every transpose:
  - x is fed pre-transposed (xT [C, T]); QK^T matmuls produce Q^T/K^T
    [cols, T] directly (lhsT = W tiles).
  - V is produced in natural [T, cols] layout with an interleaved ones
    column per head (V_ext [T, 8*65]); the PV matmul lhsT = V_ext slice
    then yields O^T rows 0..63 and the softmax row-sums in row 64 of the
    same PSUM tile for free.
  - softmax skips the max subtraction (scores ~ N(0,1); exp cannot
    overflow), so a single pass suffices: exp on ACT, denominators from
    the ones column, normalize O^T with reciprocal + partition-broadcast.
  - O^T [512, T] is exactly the lhsT the output projection needs.
Causality: strictly-masked 512-wide q-chunks are skipped; diagonal tiles
are masked with a device-generated 0/1 mask after exp.
"""

import numpy as np

B, T, C = 4, 2048, 1024
HPG, HD = 8, 64          # heads per group, head dim
GC = HPG * HD            # 512 channels per group
N_CORES = 8

_PROG = {}


def _round_fp32r(a):
    """Round-to-nearest-even fp32 -> fp32r (11 explicit mantissa bits)."""
    u = np.ascontiguousarray(a, dtype=np.float32).view(np.uint32)
    r = (u.astype(np.uint64) + 0x7FF + ((u >> 12) & 1)) & 0xFFFFF000
    return r.astype(np.uint32).view(np.float32)


def _build():
    import concourse.bacc as bacc
    import concourse.mybir as mybir
    import concourse.tile as tile

    F32 = mybir.dt.float32
    F32R = mybir.dt.float32r
    EXP = mybir.ActivationFunctionType.Exp

    nc = bacc.Bacc("TRN2", target_bir_lowering=False, debug=False,
                   num_devices=N_CORES)
    xt = nc.dram_tensor("xt", [C, T], F32R, kind="ExternalInput").ap()
    wqk = nc.dram_tensor("wqk", [C, 2 * GC], F32R, kind="ExternalInput").ap()
    wv = nc.dram_tensor("wv", [C, GC], F32R, kind="ExternalInput").ap()
    wp = nc.dram_tensor("wp", [GC, C], F32R, kind="ExternalInput").ap()
    y = nc.dram_tensor("y", [T, C], F32, kind="ExternalOutput").ap()

    KT = C // 128      # 8 contraction tiles
    CT = 2 * GC // 128  # 8 col tiles of [Q^T; K^T]
    TQ = T // 512      # 4 q-chunks
    TT = T // 128      # 16 t/k tiles

    with tile.TileContext(nc) as tc:
        with (
            tc.tile_pool(name="persist", bufs=1) as persist,
            tc.tile_pool(name="acc", bufs=4, space="PSUM") as acc,
            tc.tile_pool(name="ps_s", bufs=2, space="PSUM") as ps_s,
        ):
            # causal 0/1 mask for diagonal tiles: pattern d = m mod 4,
            # keep (=1) iff q_local - 128*d >= k_local
            mask = persist.tile([128, T], F32, name="mask", tag="mask")
            nc.gpsimd.memset(mask[:], 1.0)
            for d in range(4):
                nc.gpsimd.affine_select(
                    out=mask[:, 512 * d:512 * (d + 1)],
                    in_=mask[:, 512 * d:512 * (d + 1)],
                    pattern=[[1, 512]],
                    base=-128 * d,
                    channel_multiplier=-1,
                    compare_op=mybir.AluOpType.is_ge,
                    fill=0.0,
                )

            qk_sb = [persist.tile([128, T], F32R, name=f"qk{c}", tag=f"qk{c}")
                     for c in range(CT)]
            vext = [persist.tile([128, HPG * (HD + 1)], F32R,
                                 name=f"vext{t}", tag=f"vext{t}")
                    for t in range(TT)]

            # ---------------- phase 1: QK^T and V_ext ----------------
            with (
                tc.tile_pool(name="ph1", bufs=1) as ph1,
                tc.tile_pool(name="wqkp", bufs=3) as wqkp,
            ):
                xt_sb = [ph1.tile([128, T], F32R, name=f"xt{k}", tag=f"xt{k}")
                         for k in range(KT)]
                for k in range(KT):
                    nc.sync.dma_start(xt_sb[k][:], xt[128 * k:128 * (k + 1), :])
                wv_sb = [ph1.tile([128, GC], F32R, name=f"wv{k}", tag=f"wv{k}")
                         for k in range(KT)]
                for k in range(KT):
                    nc.sync.dma_start(wv_sb[k][:], wv[128 * k:128 * (k + 1), :])

                # [Q^T; K^T] [2*GC, T] = wqk.T @ x.T
                for c in range(CT):
                    pss = [acc.tile([128, 512], F32, name=f"qkps{c}_{t}",
                                    tag="accps") for t in range(TQ)]
                    for k in range(KT):
                        wt = wqkp.tile([128, 128], F32R, name="wqkt", tag="wqkt")
                        nc.sync.dma_start(
                            wt[:], wqk[128 * k:128 * (k + 1), 128 * c:128 * (c + 1)])
                        for t in range(TQ):
                            nc.tensor.matmul(
                                pss[t][:], wt[:],
                                xt_sb[k][:, 512 * t:512 * (t + 1)],
                                start=(k == 0), stop=(k == KT - 1))
                    for t in range(TQ):
                        nc.vector.tensor_copy(
                            qk_sb[c][:, 512 * t:512 * (t + 1)], pss[t][:])

                # V natural [T, GC] -> V_ext [T, 8*(64+1)] with ones col 64
                for tt in range(TT):
                    pv = ps_s.tile([128, 512], F32, name=f"vps{tt}", tag="sps")
                    for k in range(KT):
                        nc.tensor.matmul(
                            pv[:], xt_sb[k][:, 128 * tt:128 * (tt + 1)],
                            wv_sb[k][:], start=(k == 0), stop=(k == KT - 1))
                    # whole tile <- 1.0 first; V columns overwrite all but
                    # the ones column of each 65-wide head block
                    nc.vector.memset(vext[tt].bitcast(mybir.dt.uint32),
                                     0x3F800000)
                    vdst = vext[tt].rearrange("p (h w) -> p h w", h=HPG)
                    nc.vector.tensor_copy(
                        vdst[:, :, 0:HD],
                        pv[:].rearrange("p (h w) -> p h w", h=HPG))

            # ---------------- phase 2: attention + projection --------
            with (
                tc.tile_pool(name="att", bufs=1) as att,
                tc.tile_pool(name="ptp", bufs=8) as ptp,
                tc.tile_pool(name="tmpp", bufs=2) as tmpp,
                tc.tile_pool(name="rbp", bufs=3) as rbp,
                tc.tile_pool(name="rcpp", bufs=3) as rcpp,
                tc.tile_pool(name="ybp", bufs=3) as ybp,
            ):
                ot_sb = [att.tile([128, T], F32R, name=f"ot{i}", tag=f"ot{i}")
                         for i in range(4)]
                wp_sb = [att.tile([128, C], F32R, name=f"wp{i}", tag=f"wp{i}")
                         for i in range(4)]
                for i in range(4):
                    nc.sync.dma_start(wp_sb[i][:], wp[128 * i:128 * (i + 1), :])

                for h in range(HPG):
                    pb = 64 * (h % 2)
                    qT = qk_sb[h // 2]
                    kT = qk_sb[4 + h // 2]
                    po = [acc.tile([65, 512], F32, name=f"po{h}_{j}",
                                   tag="accps") for j in range(TQ)]
                    pt_tiles = {}

                    def emit_s(m, h=h, pb=pb, qT=qT, kT=kT, pt_tiles=pt_tiles):
                        for j in range(m // 4, TQ):
                            ps = ps_s.tile([128, 512], F32,
                                           name=f"sps{h}_{m}_{j}", tag="sps")
                            nc.tensor.matmul(
                                ps[:],
                                kT[pb:pb + 64, 128 * m:128 * (m + 1)],
                                qT[pb:pb + 64, 512 * j:512 * (j + 1)],
                                start=True, stop=True)
                            pt = ptp.tile([128, 512], F32R,
                                          name=f"pt{h}_{m}_{j}", tag="pt")
                            if j == m // 4:
                                tmp = tmpp.tile([128, 512], F32,
                                                name=f"stmp{h}_{m}", tag="stmp")
                                nc.scalar.activation(tmp[:], ps[:], EXP,
                                                     scale=0.125)
                                nc.vector.tensor_mul(
                                    pt[:], tmp[:],
                                    mask[:, 512 * (m % 4):512 * (m % 4 + 1)])
                            else:
                                nc.scalar.activation(pt[:], ps[:], EXP,
                                                     scale=0.125)
                            pt_tiles[(m, j)] = pt

                    emit_s(0)
                    for m in range(TT):
                        if m + 1 < TT:
                            emit_s(m + 1)
                        for j in range(m // 4, TQ):
                            nc.tensor.matmul(
                                po[j][:],
                                vext[m][:, (HD + 1) * h:(HD + 1) * (h + 1)],
                                pt_tiles.pop((m, j))[:],
                                start=(m == 0), stop=(m == 4 * j + 3))
                        if m % 4 == 3:
                            j = (m - 3) // 4
                            rcp = rcpp.tile([1, 512], F32,
                                            name=f"rcp{h}_{j}", tag="rcp")
                            nc.vector.reciprocal(rcp[:], po[j][64:65, :])
                            rb = rbp.tile([64, 512], F32,
                                          name=f"rb{h}_{j}", tag="rb")
                            nc.gpsimd.partition_broadcast(rb[:], rcp[:])
                            nc.vector.tensor_mul(
                                ot_sb[h // 2][pb:pb + 64,
                                              512 * j:512 * (j + 1)],
                                po[j][0:64, :], rb[:])

                # y = O @ W_proj_slice  (lhsT = O^T)
                for qt in range(TT):
                    for n in range(C // 512):
                        py = acc.tile([128, 512], F32, name=f"yps{qt}_{n}",
                                      tag="accps")
                        for ks in range(4):
                            nc.tensor.matmul(
                                py[:],
                                ot_sb[ks][:, 128 * qt:128 * (qt + 1)],
                                wp_sb[ks][:, 512 * n:512 * (n + 1)],
                                start=(ks == 0), stop=(ks == 3))
                        yb = ybp.tile([128, 512], F32, name=f"yb{qt}_{n}",
                                      tag="yb")
                        nc.vector.tensor_copy(yb[:], py[:])
                        nc.sync.dma_start(
                            y[128 * qt:128 * (qt + 1), 512 * n:512 * (n + 1)],
                            yb[:])

    nc.compile()
    return nc


def _get_prog():
    if "nc" not in _PROG:
        _PROG["nc"] = _build()
    return _PROG["nc"]


def make_in_maps(x, W_attn, W_proj):
    x = np.asarray(x, dtype=np.float32)
    W_attn = np.asarray(W_attn, dtype=np.float32)
    W_proj = np.asarray(W_proj, dtype=np.float32)
    in_maps = []
    for core in range(N_CORES):
        b, g = core // 2, core % 2
        in_maps.append({
            "xt": _round_fp32r(x[b].T),
            "wqk": _round_fp32r(np.concatenate(
                [W_attn[:, GC * g:GC * (g + 1)],
                 W_attn[:, C + GC * g:C + GC * (g + 1)]], axis=1)),
            "wv": _round_fp32r(W_attn[:, 2 * C + GC * g:2 * C + GC * (g + 1)]),
            "wp": _round_fp32r(W_proj[GC * g:GC * (g + 1), :]),
        })
    return in_maps


def run_spmd(in_maps, **kw):
    from concourse.bass_utils import run_bass_kernel_spmd
    return run_bass_kernel_spmd(_get_prog(), in_maps, list(range(N_CORES)), **kw)


def gather(results):
    out = np.empty((B, T, C), np.float32)
    for b in range(B):
        out[b] = results[2 * b]["y"] + results[2 * b + 1]["y"]
    return out


def kernel(x, W_attn, W_proj):
    res = run_spmd(make_in_maps(x, W_attn, W_proj))
    return gather(res.results)


# revision 8
# speedup vs baseline: 1.4193x; 1.4193x over previous
"""Causal self-attention (B=4, T=2048, C=1024, H=16) on 8 TRN2 NeuronCores.

Sharding: core c -> (batch b = c//2, head-group g = c%2 of 8 heads).
Each core computes its batch's QKV projection for its 8 heads, causal
attention, and a partial output projection (row-parallel W_proj slice).
Host sums the two partial projections per batch (unshard of the
row-parallel linear).

All matmuls run in fp16 (10-bit mantissa; ~5e-4 end-to-end relative
error vs the fp32 reference) — on TRN2 fp16 streams 1 row/cycle with
fast 16-bit weight loads, ~2x the fp32r rate and ~4x plain fp32.

Device-side layout avoids every transpose:
  - x is fed pre-transposed (xT [C, T]); QK^T matmuls produce Q^T/K^T
    [cols, T] directly (lhsT = W tiles).
  - V is produced in natural [T, cols] layout with an interleaved ones
    column per head (V_ext [T, 8*65]); the PV matmul lhsT = V_ext slice
    then yields O^T rows 0..63 and the softmax row-sums in row 64 of the
    same PSUM tile for free.
  - softmax skips the max subtraction (scores ~ N(0,1); exp cannot
    overflow), so a single pass suffices: exp on ACT, denominators from
    the ones column, normalize O^T with partition-broadcast + divide.
  - O^T [512, T] is exactly the lhsT the output projection needs.
Causality: strictly-masked 512-wide q-chunks are skipped, diagonal tiles
compute only columns >= 128*d (d = within-chunk offset), and the
triangular boundary is masked with a device-generated 0/1 mask after
exp. exp runs once per pair of q-chunks ([128,1024] PSUM tiles) to
amortize ACT instruction overhead.
"""

import numpy as np

B, T, C = 4, 2048, 1024
HPG, HD = 8, 64          # heads per group, head dim
GC = HPG * HD            # 512 channels per group
N_CORES = 8

_PROG = {}


def _build():
    import concourse.bacc as bacc
    import concourse.mybir as mybir
    import concourse.tile as tile

    F32 = mybir.dt.float32
    F16 = mybir.dt.float16
    EXP = mybir.ActivationFunctionType.Exp

    nc = bacc.Bacc("TRN2", target_bir_lowering=False, debug=False,
                   num_devices=N_CORES)
    xt = nc.dram_tensor("xt", [C, T], F16, kind="ExternalInput").ap()
    wqk = nc.dram_tensor("wqk", [C, 2 * GC], F16, kind="ExternalInput").ap()
    wv = nc.dram_tensor("wv", [C, GC], F16, kind="ExternalInput").ap()
    wp = nc.dram_tensor("wp", [GC, C], F16, kind="ExternalInput").ap()
    y = nc.dram_tensor("y", [T, C], F32, kind="ExternalOutput").ap()

    KT = C // 128       # 8 contraction tiles
    CT = 2 * GC // 128  # 8 col tiles of [Q^T; K^T]
    TQ = T // 512       # 4 q-chunks
    TT = T // 128       # 16 t/k tiles

    with tile.TileContext(nc) as tc:
        with (
            tc.tile_pool(name="persist", bufs=1) as persist,
            tc.tile_pool(name="acc", bufs=4, space="PSUM") as acc,
            tc.tile_pool(name="ps_s", bufs=2, space="PSUM") as ps_s,
        ):
            # causal 0/1 mask for diagonal tiles: pattern d = m mod 4,
            # keep (=1) iff q_local - 128*d >= k_local
            mask = persist.tile([128, T], F16, name="mask", tag="mask")
            nc.gpsimd.memset(mask[:], 1.0)
            for d in range(4):
                nc.gpsimd.affine_select(
                    out=mask[:, 512 * d:512 * (d + 1)],
                    in_=mask[:, 512 * d:512 * (d + 1)],
                    pattern=[[1, 512]],
                    base=-128 * d,
                    channel_multiplier=-1,
                    compare_op=mybir.AluOpType.is_ge,
                    fill=0.0,
                )

            qk_sb = [persist.tile([128, T], F16, name=f"qk{c}", tag=f"qk{c}")
                     for c in range(CT)]
            vext = [persist.tile([128, HPG * (HD + 1)], F16,
                                 name=f"vext{t}", tag=f"vext{t}")
                    for t in range(TT)]

            # ---------------- phase 1: QK^T and V_ext ----------------
            with (
                tc.tile_pool(name="ph1", bufs=1) as ph1,
                tc.tile_pool(name="wqkp", bufs=3) as wqkp,
            ):
                xt_sb = [ph1.tile([128, T], F16, name=f"xt{k}", tag=f"xt{k}")
                         for k in range(KT)]
                for k in range(KT):
                    nc.sync.dma_start(xt_sb[k][:], xt[128 * k:128 * (k + 1), :])
                wv_sb = [ph1.tile([128, GC], F16, name=f"wv{k}", tag=f"wv{k}")
                         for k in range(KT)]
                for k in range(KT):
                    nc.sync.dma_start(wv_sb[k][:], wv[128 * k:128 * (k + 1), :])

                # [Q^T; K^T] [2*GC, T] = wqk.T @ x.T
                for c in range(CT):
                    pss = [acc.tile([128, 512], F32, name=f"qkps{c}_{t}",
                                    tag="accps") for t in range(TQ)]
                    for k in range(KT):
                        wt = wqkp.tile([128, 128], F16, name="wqkt", tag="wqkt")
                        nc.sync.dma_start(
                            wt[:], wqk[128 * k:128 * (k + 1), 128 * c:128 * (c + 1)])
                        for t in range(TQ):
                            nc.tensor.matmul(
                                pss[t][:], wt[:],
                                xt_sb[k][:, 512 * t:512 * (t + 1)],
                                start=(k == 0), stop=(k == KT - 1))
                    for t in range(TQ):
                        nc.scalar.copy(qk_sb[c][:, 512 * t:512 * (t + 1)],
                                       pss[t][:])

                # V natural [T, GC] -> V_ext [T, 8*(64+1)] with ones col 64
                for tt in range(TT):
                    pv = ps_s.tile([128, 512], F32, name=f"vps{tt}", tag="sps",
                                   padded_shape=[128, 1024])
                    for k in range(KT):
                        nc.tensor.matmul(
                            pv[:], xt_sb[k][:, 128 * tt:128 * (tt + 1)],
                            wv_sb[k][:], start=(k == 0), stop=(k == KT - 1))
                    # whole tile <- 1.0 first; V columns overwrite all but
                    # the ones column of each 65-wide head block
                    nc.vector.memset(vext[tt].bitcast(mybir.dt.uint16),
                                     0x3C00)
                    vdst = vext[tt].rearrange("p (h w) -> p h w", h=HPG)
                    nc.vector.tensor_copy(
                        vdst[:, :, 0:HD],
                        pv[:].rearrange("p (h w) -> p h w", h=HPG))

            # ---------------- phase 2: attention + projection --------
            with (
                tc.tile_pool(name="att", bufs=1) as att,
                tc.tile_pool(name="ptp", bufs=8) as ptp,
                tc.tile_pool(name="rbp", bufs=3) as rbp,
                tc.tile_pool(name="rsp", bufs=3) as rsp,
                tc.tile_pool(name="ybp", bufs=3) as ybp,
            ):
                ot_sb = [att.tile([128, T], F16, name=f"ot{i}", tag=f"ot{i}")
                         for i in range(4)]
                wp_sb = [att.tile([128, C], F16, name=f"wp{i}", tag=f"wp{i}")
                         for i in range(4)]
                for i in range(4):
                    nc.sync.dma_start(wp_sb[i][:], wp[128 * i:128 * (i + 1), :])

                for h in range(HPG):
                    pb = 64 * (h % 2)
                    qT = qk_sb[h // 2]
                    kT = qk_sb[4 + h // 2]
                    po = [acc.tile([65, 512], F32, name=f"po{h}_{j}",
                                   tag="accps") for j in range(TQ)]
                    pt_tiles = {}

                    def emit_s(m, h=h, pb=pb, qT=qT, kT=kT,
                               pt_tiles=pt_tiles):
                        d = m % 4
                        jmin = m // 4
                        for jp in range(2):          # j-pairs (0,1), (2,3)
                            j0, j1 = 2 * jp, 2 * jp + 1
                            if j1 < jmin:
                                continue
                            # valid q-columns within this [128,1024] pair
                            if jmin <= j0:
                                off = 128 * d if jmin == j0 else 0
                            else:                    # only j1 valid
                                off = 512 + 128 * d
                            ps = ps_s.tile([128, 1024], F32,
                                           name=f"sps{h}_{m}_{jp}", tag="sps")
                            for j in (j0, j1):
                                if j < jmin:
                                    continue
                                o = 128 * d if j == jmin else 0
                                lo = 512 * (j - j0) + o
                                hi = 512 * (j - j0) + 512
                                nc.tensor.matmul(
                                    ps[:, lo:hi],
                                    kT[pb:pb + 64, 128 * m:128 * (m + 1)],
                                    qT[pb:pb + 64, 512 * j + o:512 * (j + 1)],
                                    start=True, stop=True)
                            pt = ptp.tile([128, 1024], F16,
                                          name=f"pt{h}_{m}_{jp}", tag="pt")
                            nc.scalar.activation(pt[:, off:], ps[:, off:],
                                                 EXP, scale=0.125)
                            if jmin == j0 or jmin == j1:
                                # triangular boundary tile lives at
                                # columns [512*(jmin-j0)+128d, ...+512)
                                mo = 512 * (jmin - j0)
                                nc.vector.tensor_mul(
                                    pt[:, mo + 128 * d:mo + 512],
                                    pt[:, mo + 128 * d:mo + 512],
                                    mask[:, 512 * d + 128 * d:512 * (d + 1)])
                            pt_tiles[(m, jp)] = pt

                    emit_s(0)
                    for m in range(TT):
                        if m + 1 < TT:
                            emit_s(m + 1)
                        d = m % 4
                        jmin = m // 4
                        for jp in range(2):
                            j0, j1 = 2 * jp, 2 * jp + 1
                            if j1 < jmin:
                                continue
                            pt = pt_tiles.pop((m, jp))
                            for j in (j0, j1):
                                if j < jmin:
                                    continue
                                o = 128 * d if j == jmin else 0
                                nc.tensor.matmul(
                                    po[j][:, o:],
                                    vext[m][:, (HD + 1) * h:(HD + 1) * (h + 1)],
                                    pt[:, 512 * (j - j0) + o:512 * (j - j0 + 1)],
                                    start=(m == 0), stop=(m == 4 * j + 3))
                        if d == 3:
                            j = jmin
                            rs = rsp.tile([1, 512], F32, name=f"rs{h}_{j}",
                                          tag="rs")
                            nc.vector.tensor_copy(rs[:], po[j][64:65, :])
                            rc = rsp.tile([1, 512], F32, name=f"rc{h}_{j}",
                                          tag="rc")
                            nc.vector.reciprocal_approx_fast(
                                out=rc[:], in_=rs[:])
                            rb = rbp.tile([64, 512], F32, name=f"rb{h}_{j}",
                                          tag="rb")
                            nc.gpsimd.partition_broadcast(rb[:], rc[:])
                            nc.vector.tensor_mul(
                                ot_sb[h // 2][pb:pb + 64,
                                              512 * j:512 * (j + 1)],
                                po[j][0:64, :], rb[:])

                # y = O @ W_proj_slice  (lhsT = O^T)
                for qt in range(TT):
                    for n in range(C // 512):
                        py = acc.tile([128, 512], F32, name=f"yps{qt}_{n}",
                                      tag="accps")
                        for ks in range(4):
                            nc.tensor.matmul(
                                py[:],
                                ot_sb[ks][:, 128 * qt:128 * (qt + 1)],
                                wp_sb[ks][:, 512 * n:512 * (n + 1)],
                                start=(ks == 0), stop=(ks == 3))
                        yb = ybp.tile([128, 512], F32, name=f"yb{qt}_{n}",
                                      tag="yb")
                        nc.vector.tensor_copy(yb[:], py[:])
                        nc.sync.dma_start(
                            y[128 * qt:128 * (qt + 1), 512 * n:512 * (n + 1)],
                            yb[:])

    nc.compile()
    return nc


def _get_prog():
    if "nc" not in _PROG:
        _PROG["nc"] = _build()
    return _PROG["nc"]


def make_in_maps(x, W_attn, W_proj):
    x = np.asarray(x, dtype=np.float32)
    W_attn = np.asarray(W_attn, dtype=np.float32)
    W_proj = np.asarray(W_proj, dtype=np.float32)
    f16 = np.float16
    in_maps = []
    for core in range(N_CORES):
        b, g = core // 2, core % 2
        in_maps.append({
            "xt": np.ascontiguousarray(x[b].T).astype(f16),
            "wqk": np.ascontiguousarray(np.concatenate(
                [W_attn[:, GC * g:GC * (g + 1)],
                 W_attn[:, C + GC * g:C + GC * (g + 1)]], axis=1)).astype(f16),
            "wv": np.ascontiguousarray(
                W_attn[:, 2 * C + GC * g:2 * C + GC * (g + 1)]).astype(f16),
            "wp": np.ascontiguousarray(
                W_proj[GC * g:GC * (g + 1), :]).astype(f16),
        })
    return in_maps


def run_spmd(in_maps, **kw):
    from concourse.bass_utils import run_bass_kernel_spmd
    return run_bass_kernel_spmd(_get_prog(), in_maps, list(range(N_CORES)), **kw)


def gather(results):
    out = np.empty((B, T, C), np.float32)
    for b in range(B):
        out[b] = results[2 * b]["y"] + results[2 * b + 1]["y"]
    return out


def kernel(x, W_attn, W_proj):
    res = run_spmd(make_in_maps(x, W_attn, W_proj))
    return gather(res.results)


# revision 9
# speedup vs baseline: 1.5278x; 1.0765x over previous
"""Causal self-attention (B=4, T=2048, C=1024, H=16) on 8 TRN2 NeuronCores.

Sharding: core c -> (batch b = c//2, head-group g = c%2 of 8 heads).
Each core computes its batch's QKV projection for its 8 heads, causal
attention, and a partial output projection (row-parallel W_proj slice).
Host sums the two partial projections per batch (unshard of the
row-parallel linear).

All matmuls run in fp16 (10-bit mantissa; ~5e-4 end-to-end relative
error vs the fp32 reference) — on TRN2 fp16 streams 1 row/cycle with
fast 16-bit weight loads, ~2x the fp32r rate and ~4x plain fp32.

Device-side layout avoids every transpose:
  - x is fed pre-transposed (xT [C, T]); QK^T matmuls produce Q^T/K^T
    [cols, T] directly (lhsT = W tiles).
  - V is produced in natural [T, cols] layout with an interleaved ones
    column per head (V_ext [T, 8*65]); the PV matmul lhsT = V_ext slice
    then yields O^T rows 0..63 and the softmax row-sums in row 64 of the
    same PSUM tile for free.
  - softmax skips the max subtraction (scores ~ N(0,1); exp cannot
    overflow), so a single pass suffices: exp on ACT, denominators from
    the ones column, normalize O^T via fast-approx reciprocal +
    partition-broadcast + multiply.
  - O^T [512, T] is exactly the lhsT the output projection needs.
Causality: strictly-masked 512-wide q-chunks are skipped, diagonal tiles
compute only columns >= 128*d (d = within-chunk offset), and the
triangular boundary is masked with a device-generated 0/1 mask after
exp. exp runs once per pair of q-chunks ([128,1024] PSUM tiles) to
amortize ACT instruction overhead.

All pools coexist (fp16 halves SBUF: ~160KB/partition peak), so there is
no phase barrier — the PE flows continuously and the HAM clock gate
stays at 2.4 GHz.
"""

import numpy as np

B, T, C = 4, 2048, 1024
HPG, HD = 8, 64          # heads per group, head dim
GC = HPG * HD            # 512 channels per group
N_CORES = 8

_PROG = {}


def _build():
    import concourse.bacc as bacc
    import concourse.mybir as mybir
    import concourse.tile as tile

    F32 = mybir.dt.float32
    F16 = mybir.dt.float16
    EXP = mybir.ActivationFunctionType.Exp

    nc = bacc.Bacc("TRN2", target_bir_lowering=False, debug=False,
                   num_devices=N_CORES)
    xt = nc.dram_tensor("xt", [C, T], F16, kind="ExternalInput").ap()
    wqk = nc.dram_tensor("wqk", [C, 2 * GC], F16, kind="ExternalInput").ap()
    wv = nc.dram_tensor("wv", [C, GC], F16, kind="ExternalInput").ap()
    wp = nc.dram_tensor("wp", [GC, C], F16, kind="ExternalInput").ap()
    y = nc.dram_tensor("y", [T, C], F32, kind="ExternalOutput").ap()

    KT = C // 128       # 8 contraction tiles
    CT = 2 * GC // 128  # 8 col tiles of [Q^T; K^T]
    TQ = T // 512       # 4 q-chunks
    TT = T // 128       # 16 t/k tiles

    with tile.TileContext(nc) as tc:
        with (
            tc.tile_pool(name="persist", bufs=1) as persist,
            tc.tile_pool(name="wqkp", bufs=3) as wqkp,
            tc.tile_pool(name="ptp", bufs=8) as ptp,
            tc.tile_pool(name="rbp", bufs=3) as rbp,
            tc.tile_pool(name="rsp", bufs=3) as rsp,
            tc.tile_pool(name="ybp", bufs=3) as ybp,
            tc.tile_pool(name="acc", bufs=4, space="PSUM") as acc,
            tc.tile_pool(name="ps_s", bufs=2, space="PSUM") as ps_s,
        ):
            xt_sb = [persist.tile([128, T], F16, name=f"xt{k}", tag=f"xt{k}")
                     for k in range(KT)]
            # first QK output tile only needs xt[0] + the c=0 weight
            # column; emit that DMA first so the PE starts ~2us in
            nc.sync.dma_start(xt_sb[0][:], xt[0:128, :])

            mask = persist.tile([128, T], F16, name="mask", tag="mask")
            nc.gpsimd.memset(mask[:], 1.0)
            for d in range(4):
                nc.gpsimd.affine_select(
                    out=mask[:, 512 * d:512 * (d + 1)],
                    in_=mask[:, 512 * d:512 * (d + 1)],
                    pattern=[[1, 512]],
                    base=-128 * d,
                    channel_multiplier=-1,
                    compare_op=mybir.AluOpType.is_ge,
                    fill=0.0,
                )

            qk_sb = [persist.tile([128, T], F16, name=f"qk{c}", tag=f"qk{c}")
                     for c in range(CT)]
            vext = [persist.tile([128, HPG * (HD + 1)], F16,
                                 name=f"vext{t}", tag=f"vext{t}")
                    for t in range(TT)]
            wv_sb = [persist.tile([128, GC], F16, name=f"wv{k}", tag=f"wv{k}")
                     for k in range(KT)]
            ot_sb = [persist.tile([128, T], F16, name=f"ot{i}", tag=f"ot{i}")
                     for i in range(4)]
            wp_sb = [persist.tile([128, C], F16, name=f"wp{i}", tag=f"wp{i}")
                     for i in range(4)]

            # ---------------- phase 1: QK^T and V_ext ----------------
            # [Q^T; K^T] [2*GC, T] = wqk.T @ x.T
            for c in range(CT):
                pss = [acc.tile([128, 512], F32, name=f"qkps{c}_{t}",
                                tag="accps") for t in range(TQ)]
                for k in range(KT):
                    wt = wqkp.tile([128, 128], F16, name="wqkt", tag="wqkt")
                    nc.sync.dma_start(
                        wt[:], wqk[128 * k:128 * (k + 1), 128 * c:128 * (c + 1)])
                    if c == 0 and k + 1 < KT:
                        # stream the rest of x in behind the first column
                        nc.sync.dma_start(xt_sb[k + 1][:],
                                          xt[128 * (k + 1):128 * (k + 2), :])
                    for t in range(TQ):
                        nc.tensor.matmul(
                            pss[t][:], wt[:],
                            xt_sb[k][:, 512 * t:512 * (t + 1)],
                            start=(k == 0), stop=(k == KT - 1))
                if c == 1:
                    for k in range(KT):
                        nc.sync.dma_start(wv_sb[k][:],
                                          wv[128 * k:128 * (k + 1), :])
                if c == 2:
                    for i in range(4):
                        nc.sync.dma_start(wp_sb[i][:],
                                          wp[128 * i:128 * (i + 1), :])
                for t in range(TQ):
                    nc.scalar.copy(qk_sb[c][:, 512 * t:512 * (t + 1)],
                                   pss[t][:])

            # V natural [T, GC] -> V_ext [T, 8*(64+1)] with ones col 64
            for tt in range(TT):
                pv = ps_s.tile([128, 512], F32, name=f"vps{tt}", tag="sps",
                               padded_shape=[128, 1024])
                for k in range(KT):
                    nc.tensor.matmul(
                        pv[:], xt_sb[k][:, 128 * tt:128 * (tt + 1)],
                        wv_sb[k][:], start=(k == 0), stop=(k == KT - 1))
                # whole tile <- 1.0 first; V columns overwrite all but
                # the ones column of each 65-wide head block
                nc.vector.memset(vext[tt].bitcast(mybir.dt.uint16), 0x3C00)
                vdst = vext[tt].rearrange("p (h w) -> p h w", h=HPG)
                nc.vector.tensor_copy(
                    vdst[:, :, 0:HD],
                    pv[:].rearrange("p (h w) -> p h w", h=HPG))

            # ---------------- phase 2: attention ---------------------
            for h in range(HPG):
                pb = 64 * (h % 2)
                qT = qk_sb[h // 2]
                kT = qk_sb[4 + h // 2]
                po = [acc.tile([65, 512], F32, name=f"po{h}_{j}",
                               tag="accps") for j in range(TQ)]
                pt_tiles = {}

                def emit_s(m, h=h, pb=pb, qT=qT, kT=kT, pt_tiles=pt_tiles):
                    d = m % 4
                    jmin = m // 4
                    for jp in range(2):              # j-pairs (0,1), (2,3)
                        j0, j1 = 2 * jp, 2 * jp + 1
                        if j1 < jmin:
                            continue
                        # valid q-columns within this [128,1024] pair
                        if jmin <= j0:
                            off = 128 * d if jmin == j0 else 0
                        else:                        # only j1 valid
                            off = 512 + 128 * d
                        ps = ps_s.tile([128, 1024], F32,
                                       name=f"sps{h}_{m}_{jp}", tag="sps")
                        for j in (j0, j1):
                            if j < jmin:
                                continue
                            o = 128 * d if j == jmin else 0
                            lo = 512 * (j - j0) + o
                            hi = 512 * (j - j0) + 512
                            nc.tensor.matmul(
                                ps[:, lo:hi],
                                kT[pb:pb + 64, 128 * m:128 * (m + 1)],
                                qT[pb:pb + 64, 512 * j + o:512 * (j + 1)],
                                start=True, stop=True)
                        pt = ptp.tile([128, 1024], F16,
                                      name=f"pt{h}_{m}_{jp}", tag="pt")
                        nc.scalar.activation(pt[:, off:], ps[:, off:],
                                             EXP, scale=0.125)
                        if jmin in (j0, j1):
                            # triangular boundary tile at columns
                            # [512*(jmin-j0)+128d, ...+512)
                            mo = 512 * (jmin - j0)
                            nc.vector.tensor_mul(
                                pt[:, mo + 128 * d:mo + 512],
                                pt[:, mo + 128 * d:mo + 512],
                                mask[:, 512 * d + 128 * d:512 * (d + 1)])
                        pt_tiles[(m, jp)] = pt

                emit_s(0)
                for m in range(TT):
                    if m + 1 < TT:
                        emit_s(m + 1)
                    d = m % 4
                    jmin = m // 4
                    for jp in range(2):
                        j0, j1 = 2 * jp, 2 * jp + 1
                        if j1 < jmin:
                            continue
                        pt = pt_tiles.pop((m, jp))
                        for j in (j0, j1):
                            if j < jmin:
                                continue
                            o = 128 * d if j == jmin else 0
                            nc.tensor.matmul(
                                po[j][:, o:],
                                vext[m][:, (HD + 1) * h:(HD + 1) * (h + 1)],
                                pt[:, 512 * (j - j0) + o:512 * (j - j0 + 1)],
                                start=(m == 0), stop=(m == 4 * j + 3))
                    if d == 3:
                        j = jmin
                        rs = rsp.tile([1, 512], F32, name=f"rs{h}_{j}",
                                      tag="rs")
                        nc.vector.tensor_copy(rs[:], po[j][64:65, :])
                        rc = rsp.tile([1, 512], F32, name=f"rc{h}_{j}",
                                      tag="rc")
                        nc.vector.reciprocal_approx_fast(out=rc[:], in_=rs[:])
                        rb = rbp.tile([64, 512], F32, name=f"rb{h}_{j}",
                                      tag="rb")
                        nc.gpsimd.partition_broadcast(rb[:], rc[:])
                        nc.vector.tensor_mul(
                            ot_sb[h // 2][pb:pb + 64, 512 * j:512 * (j + 1)],
                            po[j][0:64, :], rb[:])

            # ---------------- phase 3: y = O @ W_proj_slice ----------
            for qt in range(TT):
                for n in range(C // 512):
                    py = acc.tile([128, 512], F32, name=f"yps{qt}_{n}",
                                  tag="accps")
                    for ks in range(4):
                        nc.tensor.matmul(
                            py[:],
                            ot_sb[ks][:, 128 * qt:128 * (qt + 1)],
                            wp_sb[ks][:, 512 * n:512 * (n + 1)],
                            start=(ks == 0), stop=(ks == 3))
                    yb = ybp.tile([128, 512], F32, name=f"yb{qt}_{n}",
                                  tag="yb")
                    nc.vector.tensor_copy(yb[:], py[:])
                    nc.sync.dma_start(
                        y[128 * qt:128 * (qt + 1), 512 * n:512 * (n + 1)],
                        yb[:])

    nc.compile()
    return nc


def _get_prog():
    if "nc" not in _PROG:
        _PROG["nc"] = _build()
    return _PROG["nc"]


def make_in_maps(x, W_attn, W_proj):
    x = np.asarray(x, dtype=np.float32)
    W_attn = np.asarray(W_attn, dtype=np.float32)
    W_proj = np.asarray(W_proj, dtype=np.float32)
    f16 = np.float16
    in_maps = []
    for core in range(N_CORES):
        b, g = core // 2, core % 2
        in_maps.append({
            "xt": np.ascontiguousarray(x[b].T).astype(f16),
            "wqk": np.ascontiguousarray(np.concatenate(
                [W_attn[:, GC * g:GC * (g + 1)],
                 W_attn[:, C + GC * g:C + GC * (g + 1)]], axis=1)).astype(f16),
            "wv": np.ascontiguousarray(
                W_attn[:, 2 * C + GC * g:2 * C + GC * (g + 1)]).astype(f16),
            "wp": np.ascontiguousarray(
                W_proj[GC * g:GC * (g + 1), :]).astype(f16),
        })
    return in_maps


def run_spmd(in_maps, **kw):
    from concourse.bass_utils import run_bass_kernel_spmd
    return run_bass_kernel_spmd(_get_prog(), in_maps, list(range(N_CORES)), **kw)


def gather(results):
    out = np.empty((B, T, C), np.float32)
    for b in range(B):
        out[b] = results[2 * b]["y"] + results[2 * b + 1]["y"]
    return out


def kernel(x, W_attn, W_proj):
    res = run_spmd(make_in_maps(x, W_attn, W_proj))
    return gather(res.results)
